# revision 8
# baseline (speedup 1.0000x reference)
"""Trainium2 Bass kernel for a 2-layer BiLSTM text classifier (v2).

Computation (matches the reference):
  e = emb[x]  ->  BiLSTM1 (return sequences)  ->  BiLSTM2 (return last state)
  -> softmax(h @ Wd + bd)

Sharding: pure data-parallel over batch across 8 cores (16 rows/core),
weights replicated, no collectives.

v2 design (cost-model driven):
  * xw = x@W+b is GEMMed directly INTO the PSUM chunk tile and the
    recurrent U-matmuls accumulate on top (start=False) -- no identity
    seed matmul, no PSUM->SBUF xw copy, no bias ACT pass.
    PSUM pending-zero semantics: exactly ONE start=True per 2KB bank
    (the first GEMM matmul), everything later start=False.
  * Both directions share one PSUM chunk tile -> ONE sigmoid and ONE
    tanh per (sub-chain, step) covering f+b, and the whole elementwise
    tail is fused across dirs at 2x width.
  * Backward-direction GEMMs read eT/seqT through reversed
    (negative-stride) APs -- no reversed copies, no per-t-slice matmuls.
  * Elementwise tail uses scalar_tensor_tensor (4x_2p eligible) on
    bf16 operands; the g-gate fixup (tanh(x)=2*sigmoid(2x)-1 with
    pre-scaled weights) runs on GPSIMD to offload DVE.
  * SUBS independent sub-chains (batch split) hide the ~1.5-2us serial
    step latency (PE -> ACT -> DVE -> ACT -> DVE -> PE).
"""

import numpy as np
import ml_dtypes

import concourse.bass as bass
import concourse.mybir as mybir
import concourse.tile as tile
from concourse import bacc
from concourse.bass_utils import run_bass_kernel_spmd
from concourse.masks import make_identity

# Problem dims (hardcoded per spec)
B, T, V, D, H, C = 128, 512, 50000, 128, 256, 10
NCORES = 8
BL = B // NCORES          # 16 batch rows per core
G = 4 * H                 # 1024 gate width
NM = G // 128             # 8 gate m-tiles
CH = 8                    # scan steps per PSUM chunk
NCH = T // CH             # 64 chunks
NTOK = T * BL             # 8192 tokens per core, time-major col = t*BL + j
GCH = NTOK // 128         # 64 embedding gather chunks

import os
SUBS = int(os.environ.get("SUBS", "1"))  # batch sub-chains per core
DOUBLE_ROW = os.environ.get("DR", "1") == "1"  # 2 k-tiles per matmul
SB = BL // SUBS           # batch rows per sub-chain

F32 = mybir.dt.float32
BF16 = mybir.dt.bfloat16
I32 = mybir.dt.int32
BF = ml_dtypes.bfloat16
AF = mybir.ActivationFunctionType
ALU = mybir.AluOpType

C_BF16 = True             # keep cell state c in bf16 (2x DVE mode)
GG_ON_POOL = True         # g-gate fixup on GPSIMD instead of DVE

TRACE = False
LAST_RESULTS = None

# Keras gate order i,f,g,o (H each) -> i,f,o,g so sigmoid gates contiguous
# m-tiles: 0,1=i  2,3=f  4,5=o  6,7=g(tanh via 2*sigmoid(2x)-1, pre-scaled)
_PERM = np.concatenate(
    [np.arange(0, 2 * H), np.arange(3 * H, 4 * H), np.arange(2 * H, 3 * H)]
)


def _pack_k(w, kt):
    """[kt*128, G] -> [128, kt, G] k-tile packing (partition-major)."""
    return np.ascontiguousarray(
        w.reshape(kt, 128, w.shape[1]).transpose(1, 0, 2)
    ).astype(BF)


def _prep_weights(inputs):
    f32 = np.float32
    out = {}
    out["emb"] = np.ascontiguousarray(np.asarray(inputs["emb"], f32))
    for nm, kt in [("U1f", 2), ("U1b", 2), ("U2f", 2), ("U2b", 2),
                   ("W1f", 1), ("W1b", 1), ("W2f", 4), ("W2b", 4)]:
        w = np.asarray(inputs[nm], f32)[:, _PERM].copy()
        w[:, 3 * H:] *= 2.0
        out[nm.lower()] = _pack_k(w, kt)
    for lay in (1, 2):
        bs = []
        for dn in ("f", "b"):
            b_ = np.asarray(inputs[f"b{lay}{dn}"], f32)[_PERM].copy()
            b_[3 * H:] *= 2.0
            bs.append(b_)
        out[f"bias{lay}"] = np.concatenate(bs).reshape(1, 2 * G).astype(BF)
    wd = np.asarray(inputs["Wd"], f32)  # [2H, C]
    out["wd"] = np.ascontiguousarray(
        wd.reshape(4, 128, C).transpose(1, 0, 2)
    ).astype(BF)
    out["bd"] = np.asarray(inputs["bd"], f32).reshape(1, C).astype(BF)
    return out


def _rev(ap, dim):
    """Reverse one free dim of an AP (negative stride view)."""
    ap = ap.copy()
    st, cnt = ap.ap[dim]
    ap.ap[dim] = [-st, cnt]
    ap.offset = ap.offset + st * (cnt - 1)
    return ap


def _build(with_bias):
    nc = bacc.Bacc("TRN2", target_bir_lowering=False, debug=False,
                   num_devices=NCORES)

    emb_d = nc.dram_tensor("emb", [V, D], F32, kind="ExternalInput")
    xidx_d = nc.dram_tensor("xidx", [128, GCH], I32, kind="ExternalInput")
    wdram = {}
    for nm in ["u1f", "u1b", "u2f", "u2b"]:
        wdram[nm] = nc.dram_tensor(nm, [128, 2, G], BF16, kind="ExternalInput")
    for nm in ["w1f", "w1b"]:
        wdram[nm] = nc.dram_tensor(nm, [128, 1, G], BF16, kind="ExternalInput")
    for nm in ["w2f", "w2b"]:
        wdram[nm] = nc.dram_tensor(nm, [128, 4, G], BF16, kind="ExternalInput")
    for nm in ["bias1", "bias2"]:
        wdram[nm] = nc.dram_tensor(nm, [1, 2 * G], BF16, kind="ExternalInput")
    wdram["wd"] = nc.dram_tensor("wd", [128, 4, C], BF16, kind="ExternalInput")
    wdram["bd"] = nc.dram_tensor("bd", [1, C], BF16, kind="ExternalInput")
    out_d = nc.dram_tensor("out", [BL, C], F32, kind="ExternalOutput")

    CDT = BF16 if C_BF16 else F32

    with tile.TileContext(nc) as tc, \
         tc.tile_pool(name="const", bufs=1) as const, \
         tc.tile_pool(name="work", bufs=2) as work, \
         tc.tile_pool(name="pszz", bufs=2, space="PSUM") as pszz:

        sb = {}
        for nm, th in wdram.items():
            t_ = const.tile(list(th.shape), th.dtype, name=f"sb_{nm}",
                            tag=f"sb_{nm}")
            nc.sync.dma_start(out=t_[:], in_=th[:])
            sb[nm] = t_
        xidx = const.tile([128, GCH], I32, name="xidx_s", tag="xidx_s")
        nc.sync.dma_start(out=xidx[:], in_=xidx_d[:])

        ident = const.tile([128, 128], F32, name="ident", tag="ident")
        make_identity(nc, ident[:])
        zero_h = const.tile([128, 2, SB], BF16, name="zero_h", tag="zero_h")
        nc.vector.memset(zero_h[:], 0.0)
        ones_r = const.tile([1, CH, SB], BF16, name="ones_r", tag="ones_r")
        nc.vector.memset(ones_r[:], 1.0)
        ones_b = const.tile([1, BL], BF16, name="ones_b", tag="ones_b")
        nc.vector.memset(ones_b[:], 1.0)

        eT = const.tile([128, NTOK], BF16, name="eT", tag="eT")
        seqT = const.tile([128, 4, NTOK], BF16, name="seqT", tag="seqT")
        c_st = [const.tile([128, 2, 2, SB], CDT, name=f"c{s}", tag=f"c{s}")
                for s in range(SUBS)]

        def zz_tile(s):
            # [d, m, t, b]; per-dir slice = 8m*CH*SB*4B = 2KB = one bank
            return pszz.tile([128, 2, NM, CH, SB], F32, name=f"zz{s}",
                             tag=f"zz{s}")

        # ---- stage A: embedding gather + transpose -> eT bf16 ----
        for ch in range(GCH):
            erows = work.tile([128, D], F32, name="erows", tag="erows", bufs=3)
            nc.gpsimd.indirect_dma_start(
                out=erows[:], out_offset=None, in_=emb_d[:],
                in_offset=bass.IndirectOffsetOnAxis(ap=xidx[:, ch:ch + 1],
                                                    axis=0))
            tpz = zz_tile(0)
            tp = tpz[:].rearrange("p a b c d -> p (a b c d)")[:, 0:128]
            nc.tensor.transpose(out=tp, in_=erows[:], identity=ident[:])
            nc.vector.tensor_copy(out=eT[:, ch * 128:(ch + 1) * 128], in_=tp)

        eT3 = eT[:].rearrange("p (t j) -> p t j", t=T)

        # ---- the scan ----
        h2 = [None] * SUBS          # layer-2 recurrent h tiles per sub

        def gemm_chunk(lay, cc, s, zz):
            """xw GEMM for chunk cc, sub s, both dirs, into PSUM zz."""
            c0, c1 = cc * CH, (cc + 1) * CH
            if lay == 1:
                rf = eT3[:, c0:c1, s * SB:(s + 1) * SB]
                rb = _rev(eT3[:, T - c1:T - c0, s * SB:(s + 1) * SB], 1)
                nk, wf, wb = 1, sb["w1f"], sb["w1b"]
            else:
                nk, wf, wb = 4, sb["w2f"], sb["w2b"]
            dr = DOUBLE_ROW and nk % 2 == 0
            for d, w_ in ((0, wf), (1, wb)):
                for m in range(NM):
                    # start=True on the first matmul touching each 2KB
                    # PSUM bank (pending-zero is bank-granular)
                    st0 = (m * CH * SB * 4) % 2048 == 0
                    if dr:
                        for k2 in range(nk // 2):
                            sq = seqT[:, 2 * k2:2 * k2 + 2, :].rearrange(
                                "p a (t j) -> p a t j", t=T)
                            if d == 0:
                                rhs = sq[:, :, c0:c1, s * SB:(s + 1) * SB]
                            else:
                                rhs = _rev(sq[:, :, T - c1:T - c0,
                                              s * SB:(s + 1) * SB], 2)
                            nc.tensor.matmul(
                                zz[:, d, m, :, :],
                                lhsT=w_[:, 2 * k2:2 * k2 + 2,
                                        m * 128:(m + 1) * 128],
                                rhs=rhs, start=(k2 == 0 and st0), stop=False,
                                skip_group_check=True,
                                perf_mode=mybir.MatmulPerfMode.DoubleRow)
                        continue
                    for k in range(nk):
                        if lay == 1:
                            rhs = rf if d == 0 else rb
                        else:
                            sq = seqT[:, k, :].rearrange("p (t j) -> p t j",
                                                         t=T)
                            if d == 0:
                                rhs = sq[:, c0:c1, s * SB:(s + 1) * SB]
                            else:
                                rhs = _rev(sq[:, T - c1:T - c0,
                                              s * SB:(s + 1) * SB], 1)
                        nc.tensor.matmul(
                            zz[:, d, m, :, :],
                            lhsT=w_[:, k, m * 128:(m + 1) * 128], rhs=rhs,
                            start=(k == 0 and st0),
                            stop=False, skip_group_check=True)
                if with_bias:
                    for m in range(NM):
                        nc.tensor.matmul(
                            zz[:, d, m, :, :],
                            lhsT=sb[f"bias{lay}"][:, (d * NM + m) * 128:
                                                  (d * NM + m + 1) * 128],
                            rhs=ones_r[:], start=False, stop=False,
                            skip_group_check=True)

        def step(lay, cc, j, s, zz):
            t_f = cc * CH + j            # forward time index
            t_b = T - 1 - t_f            # backward time index
            u = sb[f"u{lay}f"], sb[f"u{lay}b"]
            # recurrent h inputs per dir: [128, 2(k), SB] APs
            if lay == 1:
                if t_f == 0:
                    hp2 = [zero_h[:], zero_h[:]]
                else:
                    cf = (t_f - 1) * BL + s * SB
                    cb = (t_b + 1) * BL + s * SB
                    hp2 = [seqT[:, 0:2, cf:cf + SB],
                           seqT[:, 2:4, cb:cb + SB]]
            else:
                if h2[s] is None:
                    hp2 = [zero_h[:], zero_h[:]]
                else:
                    hh = h2[s]
                    hp2 = [hh[:, 0], hh[:, 1]]
            for d in range(2):
                for m in range(NM):
                    if DOUBLE_ROW:
                        nc.tensor.matmul(
                            zz[:, d, m, j, :],
                            lhsT=u[d][:, :, m * 128:(m + 1) * 128],
                            rhs=hp2[d], start=False,
                            stop=(j == CH - 1 and m == NM - 1),
                            skip_group_check=True,
                            perf_mode=mybir.MatmulPerfMode.DoubleRow)
                        continue
                    for k in range(2):
                        nc.tensor.matmul(
                            zz[:, d, m, j, :],
                            lhsT=u[d][:, k, m * 128:(m + 1) * 128],
                            rhs=hp2[d][:, k, :], start=False,
                            stop=(j == CH - 1 and m == NM - 1 and k == 1),
                            skip_group_check=True)
            g = work.tile([128, 2, NM, SB], BF16, name=f"g{s}", tag=f"g{s}",
                          bufs=3)
            nc.scalar.activation(out=g[:], in_=zz[:, :, :, j, :],
                                 func=AF.Sigmoid)
            c = c_st[s]
            t2 = work.tile([128, 2, 2, SB], CDT, name=f"t2_{s}", tag=f"t2_{s}",
                           bufs=3)
            nc.vector.scalar_tensor_tensor(
                out=t2[:], in0=c[:], scalar=1.0, in1=g[:, :, 2:4, :],
                op0=ALU.mult, op1=ALU.mult)
            gg = work.tile([128, 2, 2, SB], BF16, name=f"gg{s}", tag=f"gg{s}",
                           bufs=3)
            eng = nc.gpsimd if GG_ON_POOL else nc.vector
            eng.tensor_scalar(out=gg[:], in0=g[:, :, 6:8, :],
                              scalar1=2.0, scalar2=1.0,
                              op0=ALU.mult, op1=ALU.subtract)
            t1 = work.tile([128, 2, 2, SB], CDT, name=f"t1_{s}", tag=f"t1_{s}",
                           bufs=3)
            nc.vector.scalar_tensor_tensor(
                out=t1[:], in0=gg[:], scalar=1.0, in1=g[:, :, 0:2, :],
                op0=ALU.mult, op1=ALU.mult)
            nc.vector.scalar_tensor_tensor(
                out=c[:], in0=t1[:], scalar=0.0, in1=t2[:],
                op0=ALU.add, op1=ALU.add)
            th = work.tile([128, 2, 2, SB], BF16, name=f"th{s}", tag=f"th{s}",
                           bufs=3)
            nc.scalar.activation(out=th[:], in_=c[:], func=AF.Tanh)
            if lay == 1:
                for d, tt in ((0, t_f), (1, t_b)):
                    nc.vector.scalar_tensor_tensor(
                        out=seqT[:, 2 * d:2 * d + 2,
                                 tt * BL + s * SB:tt * BL + s * SB + SB],
                        in0=g[:, d, 4:6, :], scalar=1.0, in1=th[:, d],
                        op0=ALU.mult, op1=ALU.mult)
            else:
                hn = work.tile([128, 2, 2, SB], BF16, name=f"h2_{s}",
                               tag=f"h2_{s}", bufs=3)
                nc.vector.scalar_tensor_tensor(
                    out=hn[:], in0=g[:, :, 4:6, :], scalar=1.0, in1=th[:],
                    op0=ALU.mult, op1=ALU.mult)
                h2[s] = hn

        for lay in (1, 2):
            for s in range(SUBS):
                nc.vector.memset(c_st[s][:], 0.0)
                h2[s] = None
            for cc in range(NCH):
                zzs = [zz_tile(s) for s in range(SUBS)]
                for s in range(SUBS):
                    gemm_chunk(lay, cc, s, zzs[s])
                for j in range(CH):
                    for s in range(SUBS):
                        step(lay, cc, j, s, zzs[s])

        # ---- dense + softmax ----
        pz = zz_tile(0)
        ps = pz[:].rearrange("p a b c d -> p (a b c d)")[0:BL, 0:C]
        hT = const.tile([128, 2, 2, BL], BF16, name="hT", tag="hT")
        for s in range(SUBS):
            nc.vector.tensor_copy(out=hT[:, :, :, s * SB:(s + 1) * SB],
                                  in_=h2[s][:])
        for i, (d, k) in enumerate([(0, 0), (0, 1), (1, 0), (1, 1)]):
            nc.tensor.matmul(
                ps, lhsT=hT[:, d, k, :],
                rhs=sb["wd"][:, i, :], start=(i == 0),
                stop=False, skip_group_check=True)
        nc.tensor.matmul(ps[:, :], lhsT=ones_b[:], rhs=sb["bd"][:],
                         start=False, stop=True, skip_group_check=True)
        mx = work.tile([BL, 1], F32, name="mx", tag="mx")
        nc.vector.reduce_max(out=mx[:], in_=ps, axis=mybir.AxisListType.X)
        mxn = work.tile([BL, 1], F32, name="mxn", tag="mxn")
        nc.vector.tensor_scalar_mul(mxn[:], mx[:], -1.0)
        ex = work.tile([BL, C], F32, name="ex", tag="ex")
        sm = work.tile([BL, 1], F32, name="sm", tag="sm")
        nc.scalar.activation(out=ex[:], in_=ps, func=AF.Exp,
                             bias=mxn[:, 0:1], scale=1.0, accum_out=sm[:])
        rs = work.tile([BL, 1], F32, name="rs", tag="rs")
        nc.vector.reciprocal(rs[:], sm[:])
        osm = work.tile([BL, C], F32, name="osm", tag="osm")
        nc.vector.tensor_scalar_mul(osm[:], ex[:], rs[:, 0:1])
        nc.sync.dma_start(out=out_d[:], in_=osm[:])

    nc.compile()
    return nc


_CACHE = {}


def make_in_maps(inputs):
    w = _prep_weights(inputs)
    x = np.asarray(inputs["x"], np.int32)  # [B, T]
    in_maps = []
    for core in range(NCORES):
        xc = x[core * BL:(core + 1) * BL]            # [BL, T]
        tm = np.ascontiguousarray(xc.T).reshape(-1)  # time-major [T*BL]
        xi = np.ascontiguousarray(tm.reshape(GCH, 128).T).astype(np.int32)
        m = {"xidx": xi, "emb": w["emb"]}
        for nm in ["u1f", "u1b", "u2f", "u2b", "w2f", "w2b",
                   "bias1", "bias2", "wd", "bd"]:
            m[nm] = w[nm]
        for nm in ["w1f", "w1b"]:
            m[nm] = w[nm].reshape(128, 1, G)
        in_maps.append(m)
    return in_maps


def _has_bias(inputs):
    return any(np.any(np.asarray(inputs[nm]))
               for nm in ["b1f", "b1b", "b2f", "b2b"])


def get_nc(with_bias=True):
    key = ("nc", bool(with_bias))
    if key not in _CACHE:
        _CACHE[key] = _build(bool(with_bias))
    return _CACHE[key]


def kernel(**inputs):
    global LAST_RESULTS
    nc = get_nc(_has_bias(inputs))
    in_maps = make_in_maps(inputs)
    res = run_bass_kernel_spmd(nc, in_maps, core_ids=list(range(NCORES)),
                               trace=TRACE)
    LAST_RESULTS = res
    return np.concatenate([r["out"] for r in res.results], axis=0)
